# revision 1
# baseline (speedup 1.0000x reference)
"""Trainium2 Bass kernel for nn_KANNeuron (Chebyshev-KAN neuron layer).

Math: out[b] = hw * sum_d sum_k c[d,k] * T_k(tanh(x[b,d]))

Strategy: with t = tanh(x), s = t^2 and global centering constants p, r,
the per-dim degree-8 polynomial phi_d(t) = A_d(s) + t*B_d(s) is an exact
per-dim-weighted combination of EIGHT global maps
    t, s, ts = t*s, c2 = (s+p)^2, c3 = s*c2, c4 = (c2+r)^2,
    o2 = t*c2, o3 = t*c3
(all bounded O(1) -> fp16-safe; the per-dim weights come from one shared,
well-conditioned 4x4 solve per parity and ride the PE stationary columns).

Engine split per core (batch-sharded, 2048 rows/core, dims on partitions):
  ACT:  tanh, c2 = Sq(s+p), c4 = Sq(c2+r)    (squares via per-part. bias)
  DVE:  s, ts, c3, o2, o3 (fp16 tensor_mul at 2x)
  PE:   8 maps x 8 dim-groups x 512-col chunks accumulated into PSUM with
        per-(group,map) fp16 coefficient columns; ldweights hide under the
        moving streams.
Host adds the batch-independent constant C0 and horizontal_weight.
"""

import os
from functools import lru_cache

import numpy as np

import concourse.bass as bass
import concourse.tile as tile
from concourse import mybir
from concourse.bass_utils import run_bass_kernel_spmd
from concourse.vector_clock import ScopedClock, VectorClock

# ---------------------------------------------------------------- constants
B = 16384
D = 1024
DEG = 8
NCORES = 8
BPC = B // NCORES        # 2048 batch rows per core
P = 128                  # partitions
G = D // P               # 8 dim groups
C = 512                  # batch columns per chunk (= PSUM bank fp32 width)
NCHUNK = BPC // C        # 4
FREE = G * C             # 4096 free elements per chunk tile
NMAP = 8

P_G = -0.40              # global centering shift for c2 = (s+p)^2
R_G = -0.06              # global centering shift for c4 = (c2+r)^2

F16 = mybir.dt.float16
F32 = mybir.dt.float32

LAST_EXEC_NS = None      # filled when KERNEL_PROFILE=1
LAST_PROFILE = None


# ------------------------------------------------------- walrus workaround
def _split_drain_and_barrier(self, tick_clock, wait_clock):
    """Tile's final Drain carries one sem-wait per ticked logical processor;
    walrus CoreV2/V3 codegen rejects >1 sync wait on a CTRL instruction.
    Split the waits across single-wait NOPs on the sync engine instead."""
    gc = tick_clock.global_clock
    vals = list(gc)
    for i, v in enumerate(vals):
        if v <= 0:
            continue
        sub = [0] * len(vals)
        sub[i] = v
        nop = self.nc.sync.nop(nofuse=True, hint="drain_split_wait")
        wait_clock.add_sem_waits(nop.ins, ScopedClock({None: VectorClock(sub)}))
    self.nc.sync.drain()
    self.nc.all_engine_barrier()
    assert self.sems is not None
    popped = self.nc._tile_sem_poison_stack.pop()
    assert popped is self._sem_poison
    self.nc.clear_and_free_semaphores(list(self.sems.allocated().values()))
    self.nc.all_engine_barrier()


tile.TileContext._drain_and_barrier = _split_drain_and_barrier

# This container's walrus (CoreV2/V3 codegen) supports at most ONE sync wait
# per instruction. Tile routinely attaches several. Hoist all but the last
# wait of every committed instruction onto same-engine no-fuse NOPs placed
# immediately before it (engine programs are sequential, so semantics hold).
_orig_commit_instruction = tile.TileContext._commit_instruction
_wsplit_seq = [0]


def _commit_split_waits(self, inst, lazy_reg_writes=True):
    si = getattr(inst, "sync_info", None)
    eng = getattr(inst, "engine", None)
    if (
        si is not None
        and si.on_wait is not None
        and len(si.on_wait) > 1
        and eng is not None
    ):
        waits = list(si.on_wait)
        for w in waits[:-1]:
            _wsplit_seq[0] += 1
            nop = mybir.InstNoOp(
                name=f"wsplit_{_wsplit_seq[0]}",
                engine=eng,
                bass_nofuse=True,
                sync_info=mybir.SyncInfo(on_wait=[w], on_update=[]),
            )
            _orig_commit_instruction(self, nop, lazy_reg_writes=False)
        inst.sync_info = mybir.SyncInfo(
            on_wait=[waits[-1]], on_update=list(si.on_update or [])
        )
    return _orig_commit_instruction(self, inst, lazy_reg_writes)


tile.TileContext._commit_instruction = _commit_split_waits


# --------------------------------------------- NTFF profiling hook (axon)
def _install_ntff_hook():
    """This container's trn_rl_repo lacks antenv.axon_hooks; recreate the
    ctypes NTFF hook against the loaded libaxon_pjrt.so so trace=True works."""
    import contextlib
    import ctypes
    import sys
    import types

    try:
        from antenv.axon_hooks import get_axon_ntff_profile_hook  # noqa: F401

        return
    except ImportError:
        pass

    so_path = os.environ.get("AXON_PJRT_SO", "/opt/axon/libaxon_pjrt.so")
    hook = None
    if os.path.exists(so_path):
        lib = ctypes.CDLL(so_path)
        if hasattr(lib, "axon_start_nrt_profile"):
            lib.axon_start_nrt_profile.argtypes = [
                ctypes.POINTER(ctypes.c_int64),
                ctypes.c_size_t,
            ]
            lib.axon_start_nrt_profile.restype = ctypes.c_int64
            lib.axon_stop_nrt_profile.argtypes = [ctypes.c_char_p]
            lib.axon_stop_nrt_profile.restype = ctypes.c_int64

            @contextlib.contextmanager
            def _hook(output_dir, device_ids):
                import jax

                jax.devices()
                if device_ids:
                    ids = (ctypes.c_int64 * len(device_ids))(*device_ids)
                    rc = lib.axon_start_nrt_profile(ids, len(device_ids))
                else:
                    rc = lib.axon_start_nrt_profile(None, 0)
                if rc != 0:
                    raise RuntimeError(f"axon_start_nrt_profile rc={rc}")
                try:
                    yield
                finally:
                    n = lib.axon_stop_nrt_profile(str(output_dir).encode())
                    if n < 0:
                        raise RuntimeError(f"axon_stop_nrt_profile rc={n}")

            hook = _hook

    mod = types.ModuleType("antenv.axon_hooks")
    mod.get_axon_ntff_profile_hook = lambda: hook
    mod.set_axon_ntff_profile_hook = lambda h: None
    sys.modules["antenv.axon_hooks"] = mod


_install_ntff_hook()


# Artifact upload needs bucket creds this container may not have; degrade.
import concourse.bass_utils as _bu  # noqa: E402

_orig_upload_artifacts = _bu.upload_artifacts


def _safe_upload_artifacts(tmpdir):
    try:
        return _orig_upload_artifacts(tmpdir)
    except Exception:
        return str(tmpdir)


_bu.upload_artifacts = _safe_upload_artifacts


# ------------------------------------------------------------- host helpers
def _cheb_to_monomial_matrix(deg: int) -> np.ndarray:
    """M[k, j]: T_k(t) = sum_j M[k, j] t^j (float64, exact integers)."""
    M = np.zeros((deg + 1, deg + 1))
    M[0, 0] = 1.0
    if deg >= 1:
        M[1, 1] = 1.0
    for k in range(2, deg + 1):
        M[k, 1:] += 2.0 * M[k - 1, :-1]
        M[k, :] -= M[k - 2, :]
    return M


def _basis_polys(p: float, r: float):
    """s-polynomials (degree-4 coeff vectors, index = power of s) of the
    global-shape basis: c2=(s+p)^2, sc2=s*c2, c4=(c2+r)^2."""
    c2 = np.array([p * p, 2 * p, 1.0, 0.0, 0.0])
    sc2 = np.array([0.0, p * p, 2 * p, 1.0, 0.0])
    q = np.array([p * p + r, 2 * p, 1.0])
    c4 = np.zeros(5)
    for i, qi in enumerate(q):
        for j, qj in enumerate(q):
            c4[i + j] += qi * qj
    return c2, sc2, c4


def _solve_weights(coefficients: np.ndarray):
    """Per-dim map weights from Chebyshev coefficients (float64 host math).

    Map (column) order: t, s, ts, c2, c3, c4, o2, o3. Returns
    (coef_cols [D, 8], C0)."""
    cmat = coefficients.astype(np.float64).reshape(D, DEG + 1)
    m = cmat @ _cheb_to_monomial_matrix(DEG)          # monomial in t
    alpha = m[:, 0::2]                                # [D,5] even: s^0..s^4
    beta = m[:, 1::2]                                 # [D,4] odd:  t*s^0..3

    c2p, sc2p, c4p = _basis_polys(P_G, R_G)
    # even: alpha[1..4] = w4*c4p[1:5] + d*sc2p[1:5] + e*c2p[1:5] + ws*[1,0,0,0]
    EB = np.stack([c4p[1:5], sc2p[1:5], c2p[1:5], np.array([1.0, 0, 0, 0])], 1)
    wE = np.linalg.solve(EB, alpha[:, 1:5].T).T       # [D,4]: w4, d, e, ws
    const_even = wE[:, 0] * c4p[0] + wE[:, 1] * sc2p[0] + wE[:, 2] * c2p[0]
    C0 = float((alpha[:, 0] - const_even).sum())
    # odd: beta[0..3] = f*sc2p[0:4] + g*c2p[0:4] + wts*[0,1,0,0] + wt*[1,0,0,0]
    OB = np.stack(
        [sc2p[:4], c2p[:4], np.array([0, 1.0, 0, 0]), np.array([1.0, 0, 0, 0])], 1
    )
    wO = np.linalg.solve(OB, beta.T).T                # [D,4]: f, g, wts, wt

    coef_cols = np.stack(
        [wO[:, 3], wE[:, 3], wO[:, 2], wE[:, 2],
         wE[:, 1], wE[:, 0], wO[:, 1], wO[:, 0]], axis=1
    )                                                 # [D, 8]
    return coef_cols, C0


# ------------------------------------------------------------ device program
@lru_cache(maxsize=1)
def _build_program():
    nc = bass.Bass(trn_type="TRN2", target_bir_lowering=False, num_devices=NCORES)
    xp_ext = nc.dram_tensor("xp", [P, NCHUNK * FREE], F16, kind="ExternalInput").ap()
    coef_ext = nc.dram_tensor("coef", [P, G * NMAP], F16, kind="ExternalInput").ap()
    out_ext = nc.dram_tensor("out", [1, BPC], F32, kind="ExternalOutput").ap()

    with tile.TileContext(nc) as tc:
        with (
            tc.tile_pool(name="singles", bufs=1) as singles,
            tc.tile_pool(name="xin", bufs=4) as xin,
            tc.tile_pool(name="work", bufs=2) as work,
            tc.tile_pool(name="maps", bufs=2) as maps,
            tc.tile_pool(name="ps", bufs=2, space="PSUM") as psp,
            tc.tile_pool(name="osb", bufs=2) as osb,
        ):
            coef_sb = singles.tile([P, G * NMAP], F16)
            nc.sync.dma_start(out=coef_sb[:], in_=coef_ext[:, :])
            pbias = singles.tile([P, 1], F32)
            nc.vector.memset(pbias[:], P_G)
            rbias = singles.tile([P, 1], F32)
            nc.vector.memset(rbias[:], R_G)

            for c in range(NCHUNK):
                xt = xin.tile([P, FREE], F16)
                nc.sync.dma_start(
                    out=xt[:], in_=xp_ext[:, c * FREE : (c + 1) * FREE]
                )

                t1 = work.tile([P, FREE], F16, tag="t1")
                if c == 0:
                    # split the first tanh so the t-map matmuls of groups
                    # 0-3 can issue before the second half finishes: pulls
                    # the PE start several us earlier (pipeline fill)
                    h = FREE // 2
                    nc.scalar.activation(
                        t1[:, 0:h], xt[:, 0:h], mybir.ActivationFunctionType.Tanh
                    )
                    nc.scalar.activation(
                        t1[:, h:FREE], xt[:, h:FREE],
                        mybir.ActivationFunctionType.Tanh,
                    )
                else:
                    nc.scalar.activation(
                        t1[:], xt[:], mybir.ActivationFunctionType.Tanh
                    )
                s = work.tile([P, FREE], F16, tag="s")
                c2 = work.tile([P, FREE], F16, tag="c2")
                if c == 0:
                    h = FREE // 2
                    nc.vector.tensor_mul(s[:, 0:h], t1[:, 0:h], t1[:, 0:h])
                    nc.vector.tensor_mul(s[:, h:FREE], t1[:, h:FREE], t1[:, h:FREE])
                    nc.scalar.activation(
                        c2[:, 0:h], s[:, 0:h],
                        mybir.ActivationFunctionType.Square, pbias[:, 0:1],
                    )
                    nc.scalar.activation(
                        c2[:, h:FREE], s[:, h:FREE],
                        mybir.ActivationFunctionType.Square, pbias[:, 0:1],
                    )
                else:
                    nc.vector.tensor_mul(s[:], t1[:], t1[:])
                    nc.scalar.activation(
                        c2[:], s[:], mybir.ActivationFunctionType.Square,
                        pbias[:, 0:1],
                    )
                ts = maps.tile([P, FREE], F16, tag="ts")
                nc.vector.tensor_mul(ts[:], t1[:], s[:])
                c3 = maps.tile([P, FREE], F16, tag="c3")
                nc.vector.tensor_mul(c3[:], s[:], c2[:])
                c4 = maps.tile([P, FREE], F16, tag="c4")
                nc.scalar.activation(
                    c4[:], c2[:], mybir.ActivationFunctionType.Square, rbias[:, 0:1]
                )
                o2 = maps.tile([P, FREE], F16, tag="o2")
                nc.vector.tensor_mul(o2[:], t1[:], c2[:])
                o3 = maps.tile([P, FREE], F16, tag="o3")
                nc.vector.tensor_mul(o3[:], t1[:], c3[:])

                # mover (map) order must match host coef column order
                mlist = [t1, s, ts, c2, c3, c4, o2, o3]
                ps = psp.tile([1, C], F32)
                n_mm = NMAP * G
                i = 0
                for mi, mt in enumerate(mlist):
                    for g in range(G):
                        nc.tensor.matmul(
                            ps[0:1, :],
                            coef_sb[:, g * NMAP + mi : g * NMAP + mi + 1],
                            mt[:, g * C : (g + 1) * C],
                            start=(i == 0),
                            stop=(i == n_mm - 1),
                        )
                        i += 1

                ob = osb.tile([1, C], F32)
                nc.scalar.copy(ob[:], ps[:])
                nc.sync.dma_start(out=out_ext[0:1, c * C : (c + 1) * C], in_=ob[:])

    return nc


# ------------------------------------------------------------------- kernel
def kernel(x, coefficients, horizontal_weight, degree):
    global LAST_EXEC_NS, LAST_PROFILE
    x = np.asarray(x, dtype=np.float32)
    coefficients = np.asarray(coefficients, dtype=np.float32)
    hw = float(np.asarray(horizontal_weight).reshape(-1)[0])
    deg = int(np.asarray(degree))
    assert deg == DEG and x.shape == (B, D) and coefficients.shape == (D * (DEG + 1),)

    coef_cols, C0 = _solve_weights(coefficients)
    # device layout: [p, g*8 + m] = coef_cols[g*128+p, m]
    coef_np = (
        coef_cols.reshape(G, P, NMAP).transpose(1, 0, 2).reshape(P, G * NMAP)
    ).astype(np.float16)

    # x layout per core: [p, ((c*G)+g)*C + b] = x[core*BPC + c*C + b, g*P + p]
    in_maps = []
    for core in range(NCORES):
        xc = x[core * BPC : (core + 1) * BPC, :]
        xp = (
            xc.reshape(NCHUNK, C, G, P).transpose(3, 0, 2, 1).reshape(P, NCHUNK * FREE)
        ).astype(np.float16)
        in_maps.append({"xp": xp, "coef": coef_np})

    nc = _build_program()
    trace = os.environ.get("KERNEL_PROFILE") == "1"
    res = run_bass_kernel_spmd(nc, in_maps, list(range(NCORES)), trace=trace)
    if trace:
        LAST_EXEC_NS = res.exec_time_ns
        LAST_PROFILE = res.profile_json

    out = np.empty(B, dtype=np.float32)
    for core in range(NCORES):
        out[core * BPC : (core + 1) * BPC] = res.results[core]["out"].reshape(BPC)
    return ((out + C0) * hw).astype(np.float32)



# revision 5
# speedup vs baseline: 1.1152x; 1.1152x over previous
"""Trainium2 Bass kernel for nn_KANNeuron (Chebyshev-KAN neuron layer).

Math: out[b] = hw * sum_d sum_k c[d,k] * T_k(tanh(x[b,d]))

Strategy: with t = tanh(x), s = t^2 and global centering constants p, r,
the per-dim degree-8 polynomial phi_d(t) = A_d(s) + t*B_d(s) is an exact
per-dim-weighted combination of EIGHT global maps
    t, s, ts = t*s, c2 = (s+p)^2, c3 = s*c2, c4 = (c2+r)^2,
    o2 = t*c2, o3 = t*c3
(all bounded O(1) -> fp16-safe; the per-dim weights come from one shared,
well-conditioned 4x4 solve per parity and ride the PE stationary columns).

Engine split per core (batch-sharded, 2048 rows/core, dims on partitions):
  ACT:  tanh, c2 = Sq(s+p), c4 = Sq(c2+r)    (squares via per-part. bias)
  DVE:  s, ts, c3, o2, o3 (fp16 tensor_mul at 2x)
  PE:   8 maps x 8 dim-groups x 512-col chunks accumulated into PSUM with
        per-(group,map) fp16 coefficient columns; ldweights hide under the
        moving streams.
Host adds the batch-independent constant C0 and horizontal_weight.
"""

import os
from functools import lru_cache

import numpy as np

import concourse.bass as bass
import concourse.tile as tile
from concourse import mybir
from concourse.bass_utils import run_bass_kernel_spmd
from concourse.vector_clock import ScopedClock, VectorClock

# ---------------------------------------------------------------- constants
B = 16384
D = 1024
DEG = 8
NCORES = 8
BPC = B // NCORES        # 2048 batch rows per core
P = 128                  # partitions
G = D // P               # 8 dim groups
C = 512                  # batch columns per chunk (= PSUM bank fp32 width)
NCHUNK = BPC // C        # 4
FREE = G * C             # 4096 free elements per chunk tile
NMAP = 8

P_G = -0.40              # global centering shift for c2 = (s+p)^2
R_G = -0.06              # global centering shift for c4 = (c2+r)^2

F16 = mybir.dt.float16
F32 = mybir.dt.float32

LAST_EXEC_NS = None      # filled when KERNEL_PROFILE=1
LAST_PROFILE = None


# ------------------------------------------------------- walrus workaround
def _split_drain_and_barrier(self, tick_clock, wait_clock):
    """Tile's final Drain carries one sem-wait per ticked logical processor;
    walrus CoreV2/V3 codegen rejects >1 sync wait on a CTRL instruction.
    Split the waits across single-wait NOPs on the sync engine instead."""
    gc = tick_clock.global_clock
    vals = list(gc)
    for i, v in enumerate(vals):
        if v <= 0:
            continue
        sub = [0] * len(vals)
        sub[i] = v
        nop = self.nc.sync.nop(nofuse=True, hint="drain_split_wait")
        wait_clock.add_sem_waits(nop.ins, ScopedClock({None: VectorClock(sub)}))
    self.nc.sync.drain()
    self.nc.all_engine_barrier()
    assert self.sems is not None
    popped = self.nc._tile_sem_poison_stack.pop()
    assert popped is self._sem_poison
    self.nc.clear_and_free_semaphores(list(self.sems.allocated().values()))
    self.nc.all_engine_barrier()


tile.TileContext._drain_and_barrier = _split_drain_and_barrier

# This container's walrus (CoreV2/V3 codegen) supports at most ONE sync wait
# per instruction. Tile routinely attaches several. Hoist all but the last
# wait of every committed instruction onto same-engine no-fuse NOPs placed
# immediately before it (engine programs are sequential, so semantics hold).
_orig_commit_instruction = tile.TileContext._commit_instruction
_wsplit_seq = [0]


def _commit_split_waits(self, inst, lazy_reg_writes=True):
    si = getattr(inst, "sync_info", None)
    eng = getattr(inst, "engine", None)
    if (
        si is not None
        and si.on_wait is not None
        and len(si.on_wait) > 1
        and eng is not None
    ):
        waits = list(si.on_wait)
        for w in waits[:-1]:
            _wsplit_seq[0] += 1
            nop = mybir.InstNoOp(
                name=f"wsplit_{_wsplit_seq[0]}",
                engine=eng,
                bass_nofuse=True,
                sync_info=mybir.SyncInfo(on_wait=[w], on_update=[]),
            )
            _orig_commit_instruction(self, nop, lazy_reg_writes=False)
        inst.sync_info = mybir.SyncInfo(
            on_wait=[waits[-1]], on_update=list(si.on_update or [])
        )
    return _orig_commit_instruction(self, inst, lazy_reg_writes)


tile.TileContext._commit_instruction = _commit_split_waits


# --------------------------------------------- NTFF profiling hook (axon)
def _install_ntff_hook():
    """This container's trn_rl_repo lacks antenv.axon_hooks; recreate the
    ctypes NTFF hook against the loaded libaxon_pjrt.so so trace=True works."""
    import contextlib
    import ctypes
    import sys
    import types

    try:
        from antenv.axon_hooks import get_axon_ntff_profile_hook  # noqa: F401

        return
    except ImportError:
        pass

    so_path = os.environ.get("AXON_PJRT_SO", "/opt/axon/libaxon_pjrt.so")
    hook = None
    if os.path.exists(so_path):
        lib = ctypes.CDLL(so_path)
        if hasattr(lib, "axon_start_nrt_profile"):
            lib.axon_start_nrt_profile.argtypes = [
                ctypes.POINTER(ctypes.c_int64),
                ctypes.c_size_t,
            ]
            lib.axon_start_nrt_profile.restype = ctypes.c_int64
            lib.axon_stop_nrt_profile.argtypes = [ctypes.c_char_p]
            lib.axon_stop_nrt_profile.restype = ctypes.c_int64

            @contextlib.contextmanager
            def _hook(output_dir, device_ids):
                import jax

                jax.devices()
                if device_ids:
                    ids = (ctypes.c_int64 * len(device_ids))(*device_ids)
                    rc = lib.axon_start_nrt_profile(ids, len(device_ids))
                else:
                    rc = lib.axon_start_nrt_profile(None, 0)
                if rc != 0:
                    raise RuntimeError(f"axon_start_nrt_profile rc={rc}")
                try:
                    yield
                finally:
                    n = lib.axon_stop_nrt_profile(str(output_dir).encode())
                    if n < 0:
                        raise RuntimeError(f"axon_stop_nrt_profile rc={n}")

            hook = _hook

    mod = types.ModuleType("antenv.axon_hooks")
    mod.get_axon_ntff_profile_hook = lambda: hook
    mod.set_axon_ntff_profile_hook = lambda h: None
    sys.modules["antenv.axon_hooks"] = mod


_install_ntff_hook()


# Artifact upload needs bucket creds this container may not have; degrade.
import concourse.bass_utils as _bu  # noqa: E402

_orig_upload_artifacts = _bu.upload_artifacts


def _safe_upload_artifacts(tmpdir):
    try:
        return _orig_upload_artifacts(tmpdir)
    except Exception:
        return str(tmpdir)


_bu.upload_artifacts = _safe_upload_artifacts


# ------------------------------------------------------------- host helpers
def _cheb_to_monomial_matrix(deg: int) -> np.ndarray:
    """M[k, j]: T_k(t) = sum_j M[k, j] t^j (float64, exact integers)."""
    M = np.zeros((deg + 1, deg + 1))
    M[0, 0] = 1.0
    if deg >= 1:
        M[1, 1] = 1.0
    for k in range(2, deg + 1):
        M[k, 1:] += 2.0 * M[k - 1, :-1]
        M[k, :] -= M[k - 2, :]
    return M


def _basis_polys(p: float, r: float):
    """s-polynomials (degree-4 coeff vectors, index = power of s) of the
    global-shape basis: c2=(s+p)^2, sc2=s*c2, c4=(c2+r)^2."""
    c2 = np.array([p * p, 2 * p, 1.0, 0.0, 0.0])
    sc2 = np.array([0.0, p * p, 2 * p, 1.0, 0.0])
    q = np.array([p * p + r, 2 * p, 1.0])
    c4 = np.zeros(5)
    for i, qi in enumerate(q):
        for j, qj in enumerate(q):
            c4[i + j] += qi * qj
    return c2, sc2, c4


def _solve_weights(coefficients: np.ndarray):
    """Per-dim map weights from Chebyshev coefficients (float64 host math).

    Map (column) order: t, s, ts, c2, c3, c4, o2, o3. Returns
    (coef_cols [D, 8], C0)."""
    cmat = coefficients.astype(np.float64).reshape(D, DEG + 1)
    m = cmat @ _cheb_to_monomial_matrix(DEG)          # monomial in t
    alpha = m[:, 0::2]                                # [D,5] even: s^0..s^4
    beta = m[:, 1::2]                                 # [D,4] odd:  t*s^0..3

    c2p, sc2p, c4p = _basis_polys(P_G, R_G)
    # even: alpha[1..4] = w4*c4p[1:5] + d*sc2p[1:5] + e*c2p[1:5] + ws*[1,0,0,0]
    EB = np.stack([c4p[1:5], sc2p[1:5], c2p[1:5], np.array([1.0, 0, 0, 0])], 1)
    wE = np.linalg.solve(EB, alpha[:, 1:5].T).T       # [D,4]: w4, d, e, ws
    const_even = wE[:, 0] * c4p[0] + wE[:, 1] * sc2p[0] + wE[:, 2] * c2p[0]
    C0 = float((alpha[:, 0] - const_even).sum())
    # odd: beta[0..3] = f*sc2p[0:4] + g*c2p[0:4] + wts*[0,1,0,0] + wt*[1,0,0,0]
    OB = np.stack(
        [sc2p[:4], c2p[:4], np.array([0, 1.0, 0, 0]), np.array([1.0, 0, 0, 0])], 1
    )
    wO = np.linalg.solve(OB, beta.T).T                # [D,4]: f, g, wts, wt

    coef_cols = np.stack(
        [wO[:, 3], wE[:, 3], wO[:, 2], wE[:, 2],
         wE[:, 1], wE[:, 0], wO[:, 1], wO[:, 0]], axis=1
    )                                                 # [D, 8]
    return coef_cols, C0


# ------------------------------------------------------------ device program
NCOL = int(os.environ.get("KERNEL_NCOL", "3"))   # concurrent PE col-groups
WARMUP = int(os.environ.get("KERNEL_WARMUP", "48"))  # dummy MMs to warm HAM


@lru_cache(maxsize=1)
def _build_program():
    nc = bass.Bass(trn_type="TRN2", target_bir_lowering=False, num_devices=NCORES)
    xp_ext = nc.dram_tensor("xp", [P, NCHUNK * FREE], F16, kind="ExternalInput").ap()
    coef_ext = nc.dram_tensor("coef", [P, G * NMAP], F16, kind="ExternalInput").ap()
    out_ext = nc.dram_tensor("out", [NCOL, BPC], F32, kind="ExternalOutput").ap()

    with tile.TileContext(nc) as tc:
        with (
            tc.tile_pool(name="singles", bufs=1) as singles,
            tc.tile_pool(name="xin", bufs=4) as xin,
            tc.tile_pool(name="work", bufs=2) as work,
            tc.tile_pool(name="maps", bufs=2) as maps,
            tc.tile_pool(name="ps", bufs=2, space="PSUM") as psp,
            tc.tile_pool(name="wps", bufs=1, space="PSUM") as wps,
            tc.tile_pool(name="osb", bufs=2) as osb,
        ):
            coef_sb = singles.tile([P, G * NMAP], F16)
            nc.sync.dma_start(out=coef_sb[:], in_=coef_ext[:, :])
            pbias = singles.tile([P, 1], F32)
            nc.vector.memset(pbias[:], P_G)
            rbias = singles.tile([P, 1], F32)
            nc.vector.memset(rbias[:], R_G)

            # PE warm-up: HAM un-throttles only after ~3.4us of sustained PE
            # activity; without this the first ~12 real matmuls run at 1.2GHz.
            # Dummy matmuls on a scratch sbuf tile keep the PE busy during the
            # DMA/tanh pipeline fill. Also preload the ACT spline table.
            if WARMUP:
                wsrc = singles.tile([P, 64], F16)
                nc.vector.memset(wsrc[:], 0.0)
                wpt = wps.tile([1, 64], F32)
                for i in range(WARMUP):
                    nc.tensor.matmul(
                        wpt[0:1, :], wsrc[:, 0:1], wsrc[:, 0:64],
                        start=True, stop=True,
                    )
                tpre = singles.tile([P, 1], F16)
                nc.scalar.activation(
                    tpre[:], wsrc[:, 0:1], mybir.ActivationFunctionType.Tanh
                )

            for c in range(NCHUNK):
                xt = xin.tile([P, FREE], F16)
                if c == 0:
                    # split first chunk's DMA so tanh can start sooner
                    h = FREE // 2
                    nc.sync.dma_start(out=xt[:, 0:h], in_=xp_ext[:, 0:h])
                    nc.sync.dma_start(out=xt[:, h:FREE], in_=xp_ext[:, h:FREE])
                else:
                    nc.sync.dma_start(
                        out=xt[:], in_=xp_ext[:, c * FREE : (c + 1) * FREE]
                    )

                t1 = work.tile([P, FREE], F16, tag="t1")
                if c == 0:
                    h = FREE // 2
                    nc.scalar.activation(
                        t1[:, 0:h], xt[:, 0:h], mybir.ActivationFunctionType.Tanh
                    )
                    nc.scalar.activation(
                        t1[:, h:FREE], xt[:, h:FREE],
                        mybir.ActivationFunctionType.Tanh,
                    )
                else:
                    nc.scalar.activation(
                        t1[:], xt[:], mybir.ActivationFunctionType.Tanh
                    )
                s = work.tile([P, FREE], F16, tag="s")
                c2 = work.tile([P, FREE], F16, tag="c2")
                if c == 0:
                    h = FREE // 2
                    nc.vector.tensor_mul(s[:, 0:h], t1[:, 0:h], t1[:, 0:h])
                    nc.vector.tensor_mul(s[:, h:FREE], t1[:, h:FREE], t1[:, h:FREE])
                    nc.scalar.activation(
                        c2[:, 0:h], s[:, 0:h],
                        mybir.ActivationFunctionType.Square, pbias[:, 0:1],
                    )
                    nc.scalar.activation(
                        c2[:, h:FREE], s[:, h:FREE],
                        mybir.ActivationFunctionType.Square, pbias[:, 0:1],
                    )
                else:
                    nc.vector.tensor_mul(s[:], t1[:], t1[:])
                    nc.scalar.activation(
                        c2[:], s[:], mybir.ActivationFunctionType.Square,
                        pbias[:, 0:1],
                    )
                ts = maps.tile([P, FREE], F16, tag="ts")
                nc.vector.tensor_mul(ts[:], t1[:], s[:])
                c3 = maps.tile([P, FREE], F16, tag="c3")
                nc.vector.tensor_mul(c3[:], s[:], c2[:])
                c4 = maps.tile([P, FREE], F16, tag="c4")
                nc.scalar.activation(
                    c4[:], c2[:], mybir.ActivationFunctionType.Square, rbias[:, 0:1]
                )
                o2 = maps.tile([P, FREE], F16, tag="o2")
                nc.vector.tensor_mul(o2[:], t1[:], c2[:])
                o3 = maps.tile([P, FREE], F16, tag="o3")
                nc.vector.tensor_mul(o3[:], t1[:], c3[:])

                # mover (map) order must match host coef column order.
                # The 64 [128,1]x[128,512] matmuls are spread round-robin over
                # NCOL PE column-groups via tile_position, so up to NCOL of
                # them stream concurrently on separate XBUSes (each writes its
                # own psum partition row 32*j). Host sums the NCOL rows.
                mlist = [t1, s, ts, c2, c3, c4, o2, o3]
                ps = psp.tile([P, C], F32)
                n_mm = NMAP * G
                per_col = [
                    [i for i in range(n_mm) if i % NCOL == j] for j in range(NCOL)
                ]
                first = {j: pc[0] for j, pc in enumerate(per_col)}
                last = {j: pc[-1] for j, pc in enumerate(per_col)}
                i = 0
                for mi, mt in enumerate(mlist):
                    for g in range(G):
                        j = i % NCOL
                        nc.tensor.matmul(
                            ps[32 * j : 32 * j + 1, :],
                            coef_sb[:, g * NMAP + mi : g * NMAP + mi + 1],
                            mt[:, g * C : (g + 1) * C],
                            start=(i == first[j]),
                            stop=(i == last[j]),
                            tile_position=(0, 32 * j),
                        )
                        i += 1

                ob = osb.tile([32 * (NCOL - 1) + 1, C], F32)
                nc.scalar.copy(ob[:], ps[0 : 32 * (NCOL - 1) + 1, :])
                for j in range(NCOL):
                    nc.sync.dma_start(
                        out=out_ext[j : j + 1, c * C : (c + 1) * C],
                        in_=ob[32 * j : 32 * j + 1, :],
                    )

    return nc


# ------------------------------------------------------------------- kernel
def kernel(x, coefficients, horizontal_weight, degree):
    global LAST_EXEC_NS, LAST_PROFILE
    x = np.asarray(x, dtype=np.float32)
    coefficients = np.asarray(coefficients, dtype=np.float32)
    hw = float(np.asarray(horizontal_weight).reshape(-1)[0])
    deg = int(np.asarray(degree))
    assert deg == DEG and x.shape == (B, D) and coefficients.shape == (D * (DEG + 1),)

    coef_cols, C0 = _solve_weights(coefficients)
    # device layout: [p, g*8 + m] = coef_cols[g*128+p, m]
    coef_np = (
        coef_cols.reshape(G, P, NMAP).transpose(1, 0, 2).reshape(P, G * NMAP)
    ).astype(np.float16)

    # x layout per core: [p, ((c*G)+g)*C + b] = x[core*BPC + c*C + b, g*P + p]
    in_maps = []
    for core in range(NCORES):
        xc = x[core * BPC : (core + 1) * BPC, :]
        xp = (
            xc.reshape(NCHUNK, C, G, P).transpose(3, 0, 2, 1).reshape(P, NCHUNK * FREE)
        ).astype(np.float16)
        in_maps.append({"xp": xp, "coef": coef_np})

    nc = _build_program()
    trace = os.environ.get("KERNEL_PROFILE") == "1"
    res = run_bass_kernel_spmd(nc, in_maps, list(range(NCORES)), trace=trace)
    if trace:
        LAST_EXEC_NS = res.exec_time_ns
        LAST_PROFILE = res.profile_json

    out = np.empty(B, dtype=np.float32)
    for core in range(NCORES):
        rows = res.results[core]["out"].reshape(NCOL, BPC).astype(np.float64)
        out[core * BPC : (core + 1) * BPC] = rows.sum(axis=0)
    return ((out + C0) * hw).astype(np.float32)

